# revision 5
# baseline (speedup 1.0000x reference)
"""Trainium2 Bass kernel for nn_CharacterEmbeddingLayer.

Computation (see reference):
  embed = char_vectors[char_idxs]                       # [B,S,16,64]
  per window w in (2,3,4,5):
      h_w = max_l tanh(conv_w(embed))                   # [B,S,100]
  x = concat(h_w) @ w_proj.T                            # [B,S,128]
  2x highway: x = g*relu(Wt x+bt) + (1-g)*x, g=sigmoid(Wg x+gb)

Device mapping (per core, data-parallel over batch: 8 rows => 3200 tokens):
  - one-hot built on DVE (tensor_scalar is_equal, 4x mode) from
    broadcast-DMA'd indices vs a per-partition iota column
  - embeddings in "paired" layout e_sb[128=(dim, char-parity), 8, T] via
    PE matmul cv.T @ one-hot (even chars -> partitions 0:64, odd ->
    64:128); a second shifted layout e2_sb[128, 7, T] holding odd-start
    pairs (char 2q+1 -> 0:64, char 2q+2 -> 64:128) is produced by two
    SBUF->SBUF DMAs (partition swap) so EVERY conv chunk is a full K=128
    matmul: even positions read e_sb, odd positions read e2_sb.
    ceil(w/2) matmuls per position (105/tile vs 118 for parity pairing).
  - conv positions ordered evens-then-odds per window and group emission
    interleaved across windows so the e2 DMA completes before any odd
    position's matmul issues
  - max-pool split: ACT batch-extracts some PSUM groups to a per-window
    bf16 slab, DVE grouped-reduces the others straight from PSUM into
    the same slab; slab-halving tensor_max ops (contiguous APs keep DVE
    2x mode) reduce the slab; tanh deferred past the pool (monotonic)
  - projection + highway on PE/ACT/DVE; output stored feature-major f32
    and transposed on the host.
"""

import sys

sys.path.insert(0, "/opt/trn_rl_repo")

import numpy as np
import ml_dtypes

B, S, W, D = 64, 400, 16, 64
VOCAB, HID, NF = 96, 128, 100
WINDOWS = (2, 3, 4, 5)
N_CORES = 8
TOK_PER_CORE = B * S // N_CORES  # 3200
T = 512  # max tokens per tile (PSUM bank = 512 fp32)
TILES = [(t0, min(T, TOK_PER_CORE - t0)) for t0 in range(0, TOK_PER_CORE, T)]
N_TILES = len(TILES)  # 6x512 + 1x128
GRP = 3  # conv positions per PSUM group tile (3 banks)

# which group indices (per window) are extracted by ACT (rest: DVE reduce)
ACT_GROUPS = {2: (0, 2, 4), 3: (0, 2, 4), 4: (0, 2, 3, 4), 5: (0, 2)}

_cache = {}

BF16 = ml_dtypes.bfloat16


# ---------------------------------------------------------------- schedule
def build_schedule():
    """Conv decomposition against the dual pair layouts.

    Every position l of window w is covered by ceil(w/2) K=128 matmuls:
    chunks at j=0,2,... read pair (l+j)//2 of layout (l%2); odd w adds a
    trailing single (filter col w-1 in half0, half1 zero-padded).

    units: dict key -> index; ('pair', w, j) => stacked [F_j; F_{j+1}],
           ('single', w, w-1) => F_{w-1} in half0, half1 zero.
    sched: list of (w, positions); positions ordered evens-then-odds,
           each = list of (unit_idx, layout, pair_index).
    """
    units = {}

    def uidx(key):
        if key not in units:
            units[key] = len(units)
        return units[key]

    sched = []
    for w in WINDOWS:
        L = W - w + 1
        order = [l for l in range(L) if l % 2 == 0] + [l for l in range(L) if l % 2]
        wl = []
        for l in order:
            lay = l % 2  # 0: e_sb, 1: e2_sb
            ops = []
            j = 0
            while j + 1 < w:
                c = l + j
                ops.append((uidx(("pair", w, j)), lay, (c - lay) // 2))
                j += 2
            if j < w:  # odd w: trailing single at c = l+w-1 (parity of l)
                c = l + j
                ops.append((uidx(("single", w, j)), lay, (c - lay) // 2))
            wl.append(ops)
        sched.append((w, wl))
    return units, sched


def window_groups(L):
    """Split L positions into groups of <=GRP."""
    out = []
    l = 0
    while l < L:
        n = min(GRP, L - l)
        out.append((l, n))
        l += n
    return out


# ---------------------------------------------------------------- host prep
def prep_weights(char_vectors, filts, w_proj, hw_ws, hw_bs):
    """Build the DRAM-side packed weight arrays (all tiny)."""
    units, _ = build_schedule()
    U = len(units)
    wconv = np.zeros((128, U, 128), np.float32)
    for (kind, w, j), u in units.items():
        f = filts[w].reshape(NF, w, D)  # [100, w, 64]
        wconv[0:64, u, 0:NF] = f[:, j, :].T  # [64, 100] lhsT block
        if kind == "pair":
            wconv[64:128, u, 0:NF] = f[:, j + 1, :].T
    wproj = np.zeros((128, 4, 128), np.float32)
    for c in range(4):
        wproj[0:NF, c, :] = w_proj[:, c * NF:(c + 1) * NF].T
    whw = np.zeros((128, 4, 128), np.float32)
    for i, wm in enumerate(hw_ws):  # [t_w0, g_w0, t_w1, g_w1]
        whw[:, i, :] = wm.T
    bias = np.zeros((128, 4), np.float32)
    for i, bv in enumerate(hw_bs):  # [t_b0, g_b0, t_b1, g_b1]
        bias[:, i] = bv
    return {
        "cv": np.ascontiguousarray(char_vectors.astype(BF16)),
        "wconv": np.ascontiguousarray(wconv.astype(BF16)),
        "wproj": np.ascontiguousarray(wproj.astype(BF16)),
        "whw": np.ascontiguousarray(whw.astype(BF16)),
        "bias": np.ascontiguousarray(bias),
    }


# ---------------------------------------------------------------- program
def build_program(n_tiles=N_TILES, repeat=1):  # n_tiles: prefix of TILES
    from concourse import bacc
    import concourse.mybir as mybir
    from concourse.tile import TileContext

    dt = mybir.dt
    AF = mybir.ActivationFunctionType
    AL = mybir.AluOpType
    units, sched = build_schedule()
    U = len(units)

    nc = bacc.Bacc("TRN2", target_bir_lowering=False, debug=False, num_devices=N_CORES)

    idx_d = nc.dram_tensor("idx", [16, TOK_PER_CORE], dt.int16, kind="ExternalInput")
    cv_d = nc.dram_tensor("cv", [VOCAB, D], dt.bfloat16, kind="ExternalInput")
    wconv_d = nc.dram_tensor("wconv", [128, U, 128], dt.bfloat16, kind="ExternalInput")
    wproj_d = nc.dram_tensor("wproj", [128, 4, 128], dt.bfloat16, kind="ExternalInput")
    whw_d = nc.dram_tensor("whw", [128, 4, 128], dt.bfloat16, kind="ExternalInput")
    bias_d = nc.dram_tensor("bias", [128, 4], dt.float32, kind="ExternalInput")
    out_d = nc.dram_tensor("out", [128, TOK_PER_CORE], dt.float32, kind="ExternalOutput")

    with TileContext(nc) as tc:
        with (
            tc.tile_pool(name="const", bufs=1) as cpool,
            tc.tile_pool(name="io", bufs=2) as iopool,
            tc.tile_pool(name="work", bufs=2) as wpool,
            tc.tile_pool(name="merge", bufs=4) as mpool,
            tc.tile_pool(name="pw", bufs=4) as pwpool,
            tc.tile_pool(name="grp_psum", bufs=2, space="PSUM") as gpsum,
            tc.tile_pool(name="vec_psum", bufs=2, space="PSUM") as vpsum,
        ):
            cv_sb = cpool.tile([VOCAB, D], dt.bfloat16)
            nc.sync.dma_start(cv_sb, cv_d.ap())
            wconv_sb = cpool.tile([128, U, 128], dt.bfloat16)
            nc.sync.dma_start(wconv_sb, wconv_d.ap())
            wproj_sb = cpool.tile([128, 4, 128], dt.bfloat16)
            nc.sync.dma_start(wproj_sb, wproj_d.ap())
            whw_sb = cpool.tile([128, 4, 128], dt.bfloat16)
            nc.sync.dma_start(whw_sb, whw_d.ap())
            bias_sb = cpool.tile([128, 4], dt.float32)
            nc.sync.dma_start(bias_sb, bias_d.ap())
            iota_i = cpool.tile([VOCAB, 1], dt.int32)
            nc.gpsimd.iota(iota_i, pattern=[[1, 1]], base=0, channel_multiplier=1)
            iota_sb = cpool.tile([VOCAB, 1], dt.float32)
            nc.vector.tensor_copy(iota_sb, iota_i)

            for _rep in range(repeat):
                for ti in range(n_tiles):
                    tok0, Tc = TILES[ti]
                    # ---- one-hot
                    idx_b = iopool.tile([VOCAB, 16, Tc], dt.int16, tag="idxb")
                    nc.sync.dma_start(
                        idx_b, idx_d.ap()[:, tok0:tok0 + Tc].partition_broadcast(VOCAB)
                    )
                    oh = iopool.tile([VOCAB, 16, Tc], dt.bfloat16, tag="oh")
                    nc.vector.tensor_scalar(oh, idx_b, iota_sb[:, :], None, AL.is_equal)

                    # ---- embeddings, even-pair layout
                    e_sb = iopool.tile([128, 8, Tc], dt.bfloat16, tag="esb")
                    for r0 in range(0, 8, GRP):
                        n = min(GRP, 8 - r0)
                        g = gpsum.tile([128, GRP, 512], dt.float32, tag="grp")
                        for i in range(n):
                            p = r0 + i
                            nc.tensor.matmul(g[0:64, i, 0:Tc], cv_sb, oh[:, 2 * p, :],
                                             start=True, stop=True, tile_position=(0, 0))
                            nc.tensor.matmul(g[64:128, i, 0:Tc], cv_sb, oh[:, 2 * p + 1, :],
                                             start=True, stop=True, tile_position=(0, 64))
                        nc.scalar.copy(e_sb[:, r0:r0 + n, :], g[:, 0:n, 0:Tc])

                    # ---- shifted (odd-start) pair layout via SBUF->SBUF DMA
                    # pair q holds chars (2q+1, 2q+2); slot 7's upper half
                    # (nonexistent char 16) gets finite dummy data — it is
                    # only ever multiplied by zero-padded single weights
                    e2_sb = iopool.tile([128, 8, Tc], dt.bfloat16, tag="e2sb")
                    nc.sync.dma_start(e2_sb[0:64, :, :], e_sb[64:128, 0:8, :])
                    nc.sync.dma_start(e2_sb[64:128, 0:7, :], e_sb[0:64, 1:8, :])
                    nc.sync.dma_start(e2_sb[64:128, 7:8, :], e_sb[0:64, 0:1, :])
                    elay = (e_sb, e2_sb)

                    # ---- conv + split max-pool
                    # per-window state; groups emitted interleaved across
                    # windows (2 per window, round-robin) so early groups
                    # only touch e_sb while the e2 DMA completes
                    mfin = wpool.tile([128, 4, Tc], dt.bfloat16, tag="m4")
                    win = []
                    for wi, (w, wl) in enumerate(sched):
                        groups = window_groups(len(wl))
                        acts = ACT_GROUPS[w]
                        na = sum(n for gi, (l0, n) in enumerate(groups) if gi in acts)
                        nitems = na + sum(1 for gi in range(len(groups))
                                          if gi not in acts)
                        pw = pwpool.tile([128, max(nitems, 1), Tc], dt.bfloat16,
                                         tag="pw")
                        win.append({"wi": wi, "w": w, "wl": wl, "groups": groups,
                                    "acts": acts, "pw": pw, "fill": 0})

                    emit = []
                    done = [0] * len(win)
                    while any(d < len(wv["groups"]) for d, wv in zip(done, win)):
                        for k, wv in enumerate(win):
                            take = 2 if done[k] == 0 else len(wv["groups"])
                            hi = min(done[k] + take, len(wv["groups"]))
                            for gi in range(done[k], hi):
                                emit.append((k, gi))
                            done[k] = hi

                    for k, gi in emit:
                        wv = win[k]
                        l0, n = wv["groups"][gi]
                        wl, pw = wv["wl"], wv["pw"]
                        g = gpsum.tile([128, GRP, 512], dt.float32, tag="grp")
                        for li in range(n):
                            ops = wl[l0 + li]
                            for oi, (u, lay, pidx) in enumerate(ops):
                                nc.tensor.matmul(
                                    g[:, li, 0:Tc], wconv_sb[:, u, :],
                                    elay[lay][:, pidx, :],
                                    start=(oi == 0), stop=(oi == len(ops) - 1),
                                )
                        f = wv["fill"]
                        if gi in wv["acts"]:  # ACT extract positions
                            nc.scalar.copy(pw[:, f:f + n, :], g[:, 0:n, 0:Tc])
                            wv["fill"] += n
                        else:  # DVE grouped max straight from PSUM
                            if n == 1:
                                nc.vector.tensor_copy(pw[:, f, :], g[:, 0, 0:Tc])
                            else:
                                nc.vector.tensor_reduce(
                                    pw[:, f, :],
                                    g[:, 0:n, 0:Tc].rearrange("p g t -> p t g"),
                                    axis=mybir.AxisListType.X,
                                    op=AL.max,
                                )
                            wv["fill"] += 1

                    # slab-halving max (contiguous APs keep DVE 2x mode);
                    # odd widths peel their last item into a pending list
                    # merged flat at the end
                    for wv in win:
                        pw, wi = wv["pw"], wv["wi"]
                        m = wv["fill"]
                        cur = pw
                        pend = []
                        while m > 1:
                            if m % 2:
                                pend.append(cur[:, m - 1, :])
                                m -= 1
                            h = m // 2
                            t2 = mpool.tile([128, h, Tc], dt.bfloat16, tag="tm")
                            nc.vector.tensor_max(t2, cur[:, 0:h, :], cur[:, h:m, :])
                            cur = t2
                            m = h
                        items = [cur[:, 0, :]] + pend
                        while len(items) > 2:
                            a = items.pop(0)
                            b = items.pop(0)
                            t2 = mpool.tile([128, Tc], dt.bfloat16, tag="tm1")
                            nc.vector.tensor_max(t2, a, b)
                            items.append(t2)
                        if len(items) == 2:
                            nc.vector.tensor_max(mfin[:, wi, :], items[0], items[1])
                        else:
                            nc.vector.tensor_copy(mfin[:, wi, :], items[0])

                    th = wpool.tile([128, 4, Tc], dt.bfloat16, tag="th")
                    nc.scalar.activation(th[:, :, :], mfin[:, :, :], AF.Tanh)

                    # ---- projection
                    x_ps = vpsum.tile([128, 512], dt.float32, tag="vec")
                    for c in range(4):
                        nc.tensor.matmul(x_ps[:, 0:Tc], wproj_sb[:, c, :], th[:, c, :],
                                         start=(c == 0), stop=(c == 3))
                    xs = wpool.tile([128, Tc], dt.bfloat16, tag="xs")
                    nc.scalar.copy(xs, x_ps[:, 0:Tc])

                    # ---- highway x2
                    for hl in range(2):
                        t_ps = vpsum.tile([128, 512], dt.float32, tag="vec")
                        g_ps = vpsum.tile([128, 512], dt.float32, tag="vec")
                        nc.tensor.matmul(t_ps[:, 0:Tc], whw_sb[:, 2 * hl, :], xs,
                                         start=True, stop=True)
                        nc.tensor.matmul(g_ps[:, 0:Tc], whw_sb[:, 2 * hl + 1, :], xs,
                                         start=True, stop=True)
                        tt = wpool.tile([128, Tc], dt.bfloat16, tag="tt")
                        gg = wpool.tile([128, Tc], dt.bfloat16, tag="gg")
                        nc.scalar.activation(tt, t_ps[:, 0:Tc], AF.Relu,
                                             bias=bias_sb[:, 2 * hl:2 * hl + 1], scale=1.0)
                        nc.scalar.activation(gg, g_ps[:, 0:Tc], AF.Sigmoid,
                                             bias=bias_sb[:, 2 * hl + 1:2 * hl + 2], scale=1.0)
                        dd = wpool.tile([128, Tc], dt.bfloat16, tag="dd")
                        gd = wpool.tile([128, Tc], dt.bfloat16, tag="gd")
                        nc.vector.tensor_sub(dd, tt, xs)
                        nc.vector.tensor_mul(gd, gg, dd)
                        if hl == 0:
                            xs2 = wpool.tile([128, Tc], dt.bfloat16, tag="xs")
                            nc.vector.tensor_add(xs2, xs, gd)
                            xs = xs2
                        else:
                            xf = wpool.tile([128, Tc], dt.float32, tag="xf")
                            nc.vector.tensor_add(xf, xs, gd)
                            nc.sync.dma_start(out_d.ap()[:, tok0:tok0 + Tc], xf)

    nc.compile()
    return nc


# ---------------------------------------------------------------- runner
def _make_sharded(nc):
    import jax
    from jax.sharding import Mesh, PartitionSpec
    from jax.experimental.shard_map import shard_map
    from concourse import bass2jax, mybir

    bass2jax.install_neuronx_cc_hook()
    partition_name = nc.partition_id_tensor.name if nc.partition_id_tensor else None
    in_names, out_names, out_avals = [], [], []
    for alloc in nc.m.functions[0].allocations:
        if not isinstance(alloc, mybir.MemoryLocationSet):
            continue
        name = alloc.memorylocations[0].name
        if alloc.kind == "ExternalInput":
            if name != partition_name:
                in_names.append(name)
        elif alloc.kind == "ExternalOutput":
            out_names.append(name)
            out_avals.append(
                jax.core.ShapedArray(tuple(alloc.tensor_shape), mybir.dt.np(alloc.dtype))
            )
    n_params = len(in_names)
    all_in_names = in_names + out_names
    if partition_name is not None:
        all_in_names = all_in_names + [partition_name]

    def _body(*args):
        operands = list(args)
        if partition_name is not None:
            operands.append(bass2jax.partition_id_tensor())
        outs = bass2jax._bass_exec_p.bind(
            *operands,
            out_avals=tuple(out_avals),
            in_names=tuple(all_in_names),
            out_names=tuple(out_names),
            lowering_input_output_aliases=(),
            sim_require_finite=True,
            sim_require_nnan=True,
            nc=nc,
        )
        return tuple(outs)

    devices = jax.devices()[:N_CORES]
    mesh = Mesh(np.asarray(devices), ("core",))
    n_outs = len(out_names)
    in_specs = (PartitionSpec("core"),) * (n_params + n_outs)
    out_specs = (PartitionSpec("core"),) * n_outs
    fn = jax.jit(
        shard_map(_body, mesh=mesh, in_specs=in_specs, out_specs=out_specs,
                  check_rep=False),
        keep_unused=True,
    )
    meta = {"in_names": in_names, "out_names": out_names, "out_avals": out_avals,
            "n_params": n_params}
    return fn, meta


def _get_runner():
    if "runner" not in _cache:
        nc = build_program()
        _cache["nc"] = nc
        _cache["runner"] = _make_sharded(nc)
    return _cache["runner"]


def _concat_inputs(in_maps, meta):
    concat_in = [
        np.concatenate([in_maps[c][name] for c in range(N_CORES)], axis=0)
        for name in meta["in_names"]
    ]
    concat_zeros = [
        np.zeros((N_CORES * a.shape[0], *a.shape[1:]), a.dtype)
        for a in meta["out_avals"]
    ]
    return concat_in, concat_zeros


def make_in_maps(char_idxs, char_vectors, filt2, filt3, filt4, filt5, w_proj,
                 t_w0, t_b0, t_w1, t_b1, g_w0, g_b0, g_w1, g_b1):
    wts = prep_weights(
        np.asarray(char_vectors, np.float32),
        {2: np.asarray(filt2, np.float32), 3: np.asarray(filt3, np.float32),
         4: np.asarray(filt4, np.float32), 5: np.asarray(filt5, np.float32)},
        np.asarray(w_proj, np.float32),
        [np.asarray(t_w0, np.float32), np.asarray(g_w0, np.float32),
         np.asarray(t_w1, np.float32), np.asarray(g_w1, np.float32)],
        [np.asarray(t_b0, np.float32), np.asarray(g_b0, np.float32),
         np.asarray(t_b1, np.float32), np.asarray(g_b1, np.float32)],
    )
    idx = np.asarray(char_idxs)
    assert idx.shape == (B, S, W)
    rows_per_core = B // N_CORES
    in_maps = []
    for c in range(N_CORES):
        m = dict(wts)
        m["idx"] = np.ascontiguousarray(
            idx[c * rows_per_core:(c + 1) * rows_per_core]
            .reshape(TOK_PER_CORE, 16).T.astype(np.int16)
        )
        in_maps.append(m)
    return in_maps


def kernel(**inputs) -> np.ndarray:
    in_maps = make_in_maps(**inputs)
    sharded, meta = _get_runner()
    concat_in, concat_zeros = _concat_inputs(in_maps, meta)
    out_arrs = sharded(*concat_in, *concat_zeros)
    out = np.asarray(out_arrs[0])  # [8*128, 3200]
    rows_per_core = B // N_CORES
    parts = []
    for c in range(N_CORES):
        oc = out[c * 128:(c + 1) * 128]  # [128, 3200]
        parts.append(oc.T.reshape(rows_per_core, S, HID))
    return np.ascontiguousarray(np.concatenate(parts, axis=0))


def time_kernel(inputs, repeat=(8, 25), reps=20):
    """Per-pass exec time from the slope between two repeat factors.

    Wall(R) = dispatch + hidden-overlap + R * exec, so
    exec = (wall(R2) - wall(R1)) / (R2 - R1) with interleaved sampling.
    """
    import time
    import jax
    from jax.sharding import Mesh, PartitionSpec, NamedSharding

    in_maps = make_in_maps(**inputs)
    sharded, meta = _get_runner()
    concat_in, concat_zeros = _concat_inputs(in_maps, meta)
    mesh = Mesh(np.asarray(jax.devices()[:N_CORES]), ("core",))
    shd = NamedSharding(mesh, PartitionSpec("core"))
    d_in = [jax.device_put(a, shd) for a in concat_in]
    d_zero = [jax.device_put(a, shd) for a in concat_zeros]

    r1, r2 = repeat
    fns = []
    for r in (r1, r2):
        key = ("rep", r)
        if key not in _cache:
            nc_r = build_program(repeat=r)
            _cache[key] = _make_sharded(nc_r)
        fns.append(_cache[key][0])
    fn_1, fn_2 = fns

    def timed(fn, args):
        t0 = time.perf_counter()
        out = fn(*args)
        jax.block_until_ready(out)
        return time.perf_counter() - t0

    timed(fn_1, (*d_in, *d_zero))
    timed(fn_2, (*d_in, *d_zero))
    diffs, t1s = [], []
    for _ in range(reps):
        a = timed(fn_1, (*d_in, *d_zero))
        b = timed(fn_2, (*d_in, *d_zero))
        t1s.append(a)
        diffs.append(b - a)
    diffs.sort()
    t1s.sort()
    med = diffs[len(diffs) // 2]
    per_pass = med / (r2 - r1)
    return per_pass * 1e9, t1s[len(t1s) // 2] * 1e9, med * 1e9


# revision 18
# speedup vs baseline: 1.3367x; 1.3367x over previous
"""Trainium2 Bass kernel for nn_CharacterEmbeddingLayer.

Computation (see reference):
  embed = char_vectors[char_idxs]                       # [B,S,16,64]
  per window w in (2,3,4,5):
      h_w = max_l tanh(conv_w(embed))                   # [B,S,100]
  x = concat(h_w) @ w_proj.T                            # [B,S,128]
  2x highway: x = g*relu(Wt x+bt) + (1-g)*x, g=sigmoid(Wg x+gb)

Device mapping (per core, data-parallel over batch: 8 rows => 3200 tokens):
  - one-hot built on DVE (tensor_scalar is_equal, 4x mode) from
    broadcast-DMA'd indices vs a per-partition iota column
  - embeddings in "paired" layout e_sb[128=(dim, char-parity), 8, T] via
    PE matmul cv.T @ one-hot (even chars -> partitions 0:64, odd ->
    64:128); a second shifted layout e2_sb[128, 7, T] holding odd-start
    pairs (char 2q+1 -> 0:64, char 2q+2 -> 64:128) is produced by two
    SBUF->SBUF DMAs (partition swap) so EVERY conv chunk is a full K=128
    matmul: even positions read e_sb, odd positions read e2_sb.
    ceil(w/2) matmuls per position (105/tile vs 118 for parity pairing).
  - conv positions ordered evens-then-odds per window and group emission
    interleaved across windows so the e2 DMA completes before any odd
    position's matmul issues
  - max-pool split: ACT batch-extracts some PSUM groups to a per-window
    bf16 slab, DVE grouped-reduces the others straight from PSUM into
    the same slab; slab-halving tensor_max ops (contiguous APs keep DVE
    2x mode) reduce the slab; tanh deferred past the pool (monotonic)
  - projection + highway on PE/ACT/DVE; output stored feature-major f32
    and transposed on the host.
"""

import sys

sys.path.insert(0, "/opt/trn_rl_repo")

import numpy as np
import ml_dtypes

B, S, W, D = 64, 400, 16, 64
VOCAB, HID, NF = 96, 128, 100
WINDOWS = (2, 3, 4, 5)
N_CORES = 8
TOK_PER_CORE = B * S // N_CORES  # 3200
T = 512  # max tokens per tile (PSUM bank = 512 fp32)
TILES = [(t0, min(T, TOK_PER_CORE - t0)) for t0 in range(0, TOK_PER_CORE, T)]
N_TILES = len(TILES)  # 6x512 + 1x128
GRP = 2  # conv positions per PSUM group tile (banks)
GPSUM_BUFS = 3
VPSUM_BUFS = 2  # 0: tail reuses the group pool
MERGE_BUFS = 4
PW_BUFS = 4

# which group indices (per window) are extracted by ACT (rest: DVE reduce),
# keyed by GRP; tuned to keep ~33 ACT / ~21 DVE positions per tile
ACT_GROUPS_BY_GRP = {
    3: {2: (0, 2, 4), 3: (0, 2, 4), 4: (0, 2, 3, 4), 5: (0, 2)},
    4: {2: (0, 2), 3: (0, 2), 4: (0, 2, 3), 5: (0, 2)},
    2: {2: (0, 2, 4, 6), 3: (0, 2, 4, 6), 4: (0, 2, 3, 4, 6), 5: (0, 1, 2, 4)},
}

_cache = {}

BF16 = ml_dtypes.bfloat16


# ---------------------------------------------------------------- schedule
def build_schedule():
    """Conv decomposition against the dual pair layouts.

    Every position l of window w is covered by ceil(w/2) K=128 matmuls:
    chunks at j=0,2,... read pair (l+j)//2 of layout (l%2); odd w adds a
    trailing single (filter col w-1 in half0, half1 zero-padded).

    units: dict key -> index; ('pair', w, j) => stacked [F_j; F_{j+1}],
           ('single', w, w-1) => F_{w-1} in half0, half1 zero.
    sched: list of (w, positions); positions ordered evens-then-odds,
           each = list of (unit_idx, layout, pair_index).
    """
    units = {}

    def uidx(key):
        if key not in units:
            units[key] = len(units)
        return units[key]

    sched = []
    for w in WINDOWS:
        L = W - w + 1
        order = [l for l in range(L) if l % 2 == 0] + [l for l in range(L) if l % 2]
        wl = []
        for l in order:
            lay = l % 2  # 0: e_sb, 1: e2_sb
            ops = []
            j = 0
            while j + 1 < w:
                c = l + j
                ops.append((uidx(("pair", w, j)), lay, (c - lay) // 2))
                j += 2
            if j < w:  # odd w: trailing single at c = l+w-1 (parity of l)
                c = l + j
                ops.append((uidx(("single", w, j)), lay, (c - lay) // 2))
            wl.append(ops)
        sched.append((w, wl))
    return units, sched


def window_groups(L):
    """Split L positions into groups of <=GRP."""
    out = []
    l = 0
    while l < L:
        n = min(GRP, L - l)
        out.append((l, n))
        l += n
    return out


# ---------------------------------------------------------------- host prep
def prep_weights(char_vectors, filts, w_proj, hw_ws, hw_bs):
    """Build the DRAM-side packed weight arrays (all tiny)."""
    units, _ = build_schedule()
    U = len(units)
    wconv = np.zeros((128, U, 128), np.float32)
    for (kind, w, j), u in units.items():
        f = filts[w].reshape(NF, w, D)  # [100, w, 64]
        wconv[0:64, u, 0:NF] = f[:, j, :].T  # [64, 100] lhsT block
        if kind == "pair":
            wconv[64:128, u, 0:NF] = f[:, j + 1, :].T
    wproj = np.zeros((128, 4, 128), np.float32)
    for c in range(4):
        wproj[0:NF, c, :] = w_proj[:, c * NF:(c + 1) * NF].T
    whw = np.zeros((128, 4, 128), np.float32)
    for i, wm in enumerate(hw_ws):  # [t_w0, g_w0, t_w1, g_w1]
        whw[:, i, :] = wm.T
    bias = np.zeros((128, 4), np.float32)
    for i, bv in enumerate(hw_bs):  # [t_b0, g_b0, t_b1, g_b1]
        bias[:, i] = bv
    return {
        "cv": np.ascontiguousarray(char_vectors.astype(BF16)),
        "wconv": np.ascontiguousarray(wconv.astype(BF16)),
        "wproj": np.ascontiguousarray(wproj.astype(BF16)),
        "whw": np.ascontiguousarray(whw.astype(BF16)),
        "bias": np.ascontiguousarray(bias),
    }


# ---------------------------------------------------------------- program
def build_program(n_tiles=N_TILES, repeat=1):  # n_tiles: prefix of TILES
    from concourse import bacc
    import concourse.mybir as mybir
    from concourse.tile import TileContext

    dt = mybir.dt
    AF = mybir.ActivationFunctionType
    AL = mybir.AluOpType
    units, sched = build_schedule()
    U = len(units)

    nc = bacc.Bacc("TRN2", target_bir_lowering=False, debug=False, num_devices=N_CORES)

    idx_d = nc.dram_tensor("idx", [16, TOK_PER_CORE], dt.int16, kind="ExternalInput")
    cv_d = nc.dram_tensor("cv", [VOCAB, D], dt.bfloat16, kind="ExternalInput")
    wconv_d = nc.dram_tensor("wconv", [128, U, 128], dt.bfloat16, kind="ExternalInput")
    wproj_d = nc.dram_tensor("wproj", [128, 4, 128], dt.bfloat16, kind="ExternalInput")
    whw_d = nc.dram_tensor("whw", [128, 4, 128], dt.bfloat16, kind="ExternalInput")
    bias_d = nc.dram_tensor("bias", [128, 4], dt.float32, kind="ExternalInput")
    out_d = nc.dram_tensor("out", [128, TOK_PER_CORE], dt.float32, kind="ExternalOutput")

    with TileContext(nc) as tc:
        with (
            tc.tile_pool(name="const", bufs=1) as cpool,
            tc.tile_pool(name="io", bufs=2) as iopool,
            tc.tile_pool(name="work", bufs=2) as wpool,
            tc.tile_pool(name="merge", bufs=MERGE_BUFS) as mpool,
            tc.tile_pool(name="pw", bufs=PW_BUFS) as pwpool,
            tc.tile_pool(name="grp_psum", bufs=GPSUM_BUFS, space="PSUM") as gpsum,
            tc.tile_pool(name="vec_psum", bufs=max(VPSUM_BUFS, 1), space="PSUM") as vpsum,
        ):
            # vec banks come from their own pool (1 bank each), or
            # (VPSUM_BUFS=0) borrow slots of a full group tile so all 8
            # banks serve the conv pipeline
            def vec_banks(width):
                if VPSUM_BUFS:
                    return [vpsum.tile([128, 512], dt.float32, tag="vec",
                                       name="vps")
                            for _ in range(width)]
                t = gpsum.tile([128, GRP, 512], dt.float32, tag="grp",
                               name="vgrp")
                return [t[:, i, :] for i in range(width)]
            cv_sb = cpool.tile([VOCAB, D], dt.bfloat16)
            nc.sync.dma_start(cv_sb, cv_d.ap())
            wconv_sb = cpool.tile([128, U, 128], dt.bfloat16)
            nc.sync.dma_start(wconv_sb, wconv_d.ap())
            wproj_sb = cpool.tile([128, 4, 128], dt.bfloat16)
            nc.sync.dma_start(wproj_sb, wproj_d.ap())
            whw_sb = cpool.tile([128, 4, 128], dt.bfloat16)
            nc.sync.dma_start(whw_sb, whw_d.ap())
            bias_sb = cpool.tile([128, 4], dt.float32)
            nc.sync.dma_start(bias_sb, bias_d.ap())
            iota_i = cpool.tile([VOCAB, 1], dt.int32)
            nc.gpsimd.iota(iota_i, pattern=[[1, 1]], base=0, channel_multiplier=1)
            iota_sb = cpool.tile([VOCAB, 1], dt.float32)
            nc.vector.tensor_copy(iota_sb, iota_i)

            for _rep in range(repeat):
                for ti in range(n_tiles):
                    tok0, Tc = TILES[ti]
                    # ---- one-hot
                    idx_b = iopool.tile([VOCAB, 16, Tc], dt.int16, tag="idxb")
                    nc.sync.dma_start(
                        idx_b, idx_d.ap()[:, tok0:tok0 + Tc].partition_broadcast(VOCAB)
                    )
                    oh = iopool.tile([VOCAB, 16, Tc], dt.bfloat16, tag="oh")
                    nc.vector.tensor_scalar(oh, idx_b, iota_sb[:, :], None, AL.is_equal)

                    # ---- embeddings, even-pair layout
                    e_sb = iopool.tile([128, 8, Tc], dt.bfloat16, tag="esb")
                    for r0 in range(0, 8, GRP):
                        n = min(GRP, 8 - r0)
                        g = gpsum.tile([128, GRP, 512], dt.float32, tag="grp")
                        for i in range(n):
                            p = r0 + i
                            nc.tensor.matmul(g[0:64, i, 0:Tc], cv_sb, oh[:, 2 * p, :],
                                             start=True, stop=True, tile_position=(0, 0))
                            nc.tensor.matmul(g[64:128, i, 0:Tc], cv_sb, oh[:, 2 * p + 1, :],
                                             start=True, stop=True, tile_position=(0, 64))
                        nc.scalar.copy(e_sb[:, r0:r0 + n, :], g[:, 0:n, 0:Tc])

                    # ---- shifted (odd-start) pair layout via SBUF->SBUF DMA
                    # pair q holds chars (2q+1, 2q+2); slot 7's upper half
                    # (nonexistent char 16) gets finite dummy data — it is
                    # only ever multiplied by zero-padded single weights
                    e2_sb = iopool.tile([128, 8, Tc], dt.bfloat16, tag="e2sb")
                    nc.sync.dma_start(e2_sb[0:64, :, :], e_sb[64:128, 0:8, :])
                    nc.sync.dma_start(e2_sb[64:128, 0:7, :], e_sb[0:64, 1:8, :])
                    nc.sync.dma_start(e2_sb[64:128, 7:8, :], e_sb[0:64, 0:1, :])
                    elay = (e_sb, e2_sb)

                    # ---- conv + split max-pool
                    # per-window state; groups emitted interleaved across
                    # windows (2 per window, round-robin) so early groups
                    # only touch e_sb while the e2 DMA completes
                    mfin = wpool.tile([128, 4, Tc], dt.bfloat16, tag="m4")
                    win = []
                    for wi, (w, wl) in enumerate(sched):
                        groups = window_groups(len(wl))
                        acts = ACT_GROUPS_BY_GRP[GRP][w]
                        na = sum(n for gi, (l0, n) in enumerate(groups) if gi in acts)
                        nitems = na + sum(1 for gi in range(len(groups))
                                          if gi not in acts)
                        pw = pwpool.tile([128, max(nitems, 1), Tc], dt.bfloat16,
                                         tag="pw")
                        win.append({"wi": wi, "w": w, "wl": wl, "groups": groups,
                                    "acts": acts, "pw": pw, "fill": 0})

                    emit = []
                    done = [0] * len(win)
                    while any(d < len(wv["groups"]) for d, wv in zip(done, win)):
                        for k, wv in enumerate(win):
                            take = 2 if done[k] == 0 else len(wv["groups"])
                            hi = min(done[k] + take, len(wv["groups"]))
                            for gi in range(done[k], hi):
                                emit.append((k, gi))
                            done[k] = hi

                    for k, gi in emit:
                        wv = win[k]
                        l0, n = wv["groups"][gi]
                        wl, pw = wv["wl"], wv["pw"]
                        g = gpsum.tile([128, GRP, 512], dt.float32, tag="grp")
                        for li in range(n):
                            ops = wl[l0 + li]
                            for oi, (u, lay, pidx) in enumerate(ops):
                                nc.tensor.matmul(
                                    g[:, li, 0:Tc], wconv_sb[:, u, :],
                                    elay[lay][:, pidx, :],
                                    start=(oi == 0), stop=(oi == len(ops) - 1),
                                )
                        f = wv["fill"]
                        if gi in wv["acts"]:  # ACT extract positions
                            nc.scalar.copy(pw[:, f:f + n, :], g[:, 0:n, 0:Tc])
                            wv["fill"] += n
                        else:  # DVE grouped max straight from PSUM
                            if n == 1:
                                nc.vector.tensor_copy(pw[:, f, :], g[:, 0, 0:Tc])
                            else:
                                nc.vector.tensor_reduce(
                                    pw[:, f, :],
                                    g[:, 0:n, 0:Tc].rearrange("p g t -> p t g"),
                                    axis=mybir.AxisListType.X,
                                    op=AL.max,
                                )
                            wv["fill"] += 1

                    # slab-halving max (contiguous APs keep DVE 2x mode);
                    # odd widths peel their last item into a pending list
                    # merged flat at the end
                    for wv in win:
                        pw, wi = wv["pw"], wv["wi"]
                        m = wv["fill"]
                        cur = pw
                        pend = []
                        while m > 1:
                            if m % 2:
                                pend.append(cur[:, m - 1, :])
                                m -= 1
                            h = m // 2
                            t2 = mpool.tile([128, h, Tc], dt.bfloat16, tag="tm")
                            nc.vector.tensor_max(t2, cur[:, 0:h, :], cur[:, h:m, :])
                            cur = t2
                            m = h
                        items = [cur[:, 0, :]] + pend
                        while len(items) > 2:
                            a = items.pop(0)
                            b = items.pop(0)
                            t2 = mpool.tile([128, Tc], dt.bfloat16, tag="tm1")
                            nc.vector.tensor_max(t2, a, b)
                            items.append(t2)
                        if len(items) == 2:
                            nc.vector.tensor_max(mfin[:, wi, :], items[0], items[1])
                        else:
                            nc.vector.tensor_copy(mfin[:, wi, :], items[0])

                    th = wpool.tile([128, 4, Tc], dt.bfloat16, tag="th")
                    nc.scalar.activation(th[:, :, :], mfin[:, :, :], AF.Tanh)

                    # ---- projection
                    [x_ps] = vec_banks(1)
                    for c in range(4):
                        nc.tensor.matmul(x_ps[:, 0:Tc], wproj_sb[:, c, :], th[:, c, :],
                                         start=(c == 0), stop=(c == 3))
                    xs = wpool.tile([128, Tc], dt.bfloat16, tag="xs")
                    nc.scalar.copy(xs, x_ps[:, 0:Tc])

                    # ---- highway x2
                    for hl in range(2):
                        t_ps, g_ps = vec_banks(2)
                        nc.tensor.matmul(t_ps[:, 0:Tc], whw_sb[:, 2 * hl, :], xs,
                                         start=True, stop=True)
                        nc.tensor.matmul(g_ps[:, 0:Tc], whw_sb[:, 2 * hl + 1, :], xs,
                                         start=True, stop=True)
                        tt = wpool.tile([128, Tc], dt.bfloat16, tag="tt")
                        gg = wpool.tile([128, Tc], dt.bfloat16, tag="gg")
                        nc.scalar.activation(tt, t_ps[:, 0:Tc], AF.Relu,
                                             bias=bias_sb[:, 2 * hl:2 * hl + 1], scale=1.0)
                        nc.scalar.activation(gg, g_ps[:, 0:Tc], AF.Sigmoid,
                                             bias=bias_sb[:, 2 * hl + 1:2 * hl + 2], scale=1.0)
                        dd = wpool.tile([128, Tc], dt.bfloat16, tag="dd")
                        gd = wpool.tile([128, Tc], dt.bfloat16, tag="gd")
                        nc.vector.tensor_sub(dd, tt, xs)
                        nc.vector.tensor_mul(gd, gg, dd)
                        if hl == 0:
                            xs2 = wpool.tile([128, Tc], dt.bfloat16, tag="xs")
                            nc.vector.tensor_add(xs2, xs, gd)
                            xs = xs2
                        else:
                            xf = wpool.tile([128, Tc], dt.float32, tag="xf")
                            nc.vector.tensor_add(xf, xs, gd)
                            nc.sync.dma_start(out_d.ap()[:, tok0:tok0 + Tc], xf)

    nc.compile()
    return nc


# ---------------------------------------------------------------- runner
def _make_sharded(nc):
    import jax
    from jax.sharding import Mesh, PartitionSpec
    from jax.experimental.shard_map import shard_map
    from concourse import bass2jax, mybir

    bass2jax.install_neuronx_cc_hook()
    partition_name = nc.partition_id_tensor.name if nc.partition_id_tensor else None
    in_names, out_names, out_avals = [], [], []
    for alloc in nc.m.functions[0].allocations:
        if not isinstance(alloc, mybir.MemoryLocationSet):
            continue
        name = alloc.memorylocations[0].name
        if alloc.kind == "ExternalInput":
            if name != partition_name:
                in_names.append(name)
        elif alloc.kind == "ExternalOutput":
            out_names.append(name)
            out_avals.append(
                jax.core.ShapedArray(tuple(alloc.tensor_shape), mybir.dt.np(alloc.dtype))
            )
    n_params = len(in_names)
    all_in_names = in_names + out_names
    if partition_name is not None:
        all_in_names = all_in_names + [partition_name]

    def _body(*args):
        operands = list(args)
        if partition_name is not None:
            operands.append(bass2jax.partition_id_tensor())
        outs = bass2jax._bass_exec_p.bind(
            *operands,
            out_avals=tuple(out_avals),
            in_names=tuple(all_in_names),
            out_names=tuple(out_names),
            lowering_input_output_aliases=(),
            sim_require_finite=True,
            sim_require_nnan=True,
            nc=nc,
        )
        return tuple(outs)

    devices = jax.devices()[:N_CORES]
    mesh = Mesh(np.asarray(devices), ("core",))
    n_outs = len(out_names)
    in_specs = (PartitionSpec("core"),) * (n_params + n_outs)
    out_specs = (PartitionSpec("core"),) * n_outs
    fn = jax.jit(
        shard_map(_body, mesh=mesh, in_specs=in_specs, out_specs=out_specs,
                  check_rep=False),
        keep_unused=True,
    )
    meta = {"in_names": in_names, "out_names": out_names, "out_avals": out_avals,
            "n_params": n_params}
    return fn, meta


def _get_runner():
    if "runner" not in _cache:
        nc = build_program()
        _cache["nc"] = nc
        _cache["runner"] = _make_sharded(nc)
    return _cache["runner"]


def _concat_inputs(in_maps, meta):
    concat_in = [
        np.concatenate([in_maps[c][name] for c in range(N_CORES)], axis=0)
        for name in meta["in_names"]
    ]
    concat_zeros = [
        np.zeros((N_CORES * a.shape[0], *a.shape[1:]), a.dtype)
        for a in meta["out_avals"]
    ]
    return concat_in, concat_zeros


def make_in_maps(char_idxs, char_vectors, filt2, filt3, filt4, filt5, w_proj,
                 t_w0, t_b0, t_w1, t_b1, g_w0, g_b0, g_w1, g_b1):
    wts = prep_weights(
        np.asarray(char_vectors, np.float32),
        {2: np.asarray(filt2, np.float32), 3: np.asarray(filt3, np.float32),
         4: np.asarray(filt4, np.float32), 5: np.asarray(filt5, np.float32)},
        np.asarray(w_proj, np.float32),
        [np.asarray(t_w0, np.float32), np.asarray(g_w0, np.float32),
         np.asarray(t_w1, np.float32), np.asarray(g_w1, np.float32)],
        [np.asarray(t_b0, np.float32), np.asarray(g_b0, np.float32),
         np.asarray(t_b1, np.float32), np.asarray(g_b1, np.float32)],
    )
    idx = np.asarray(char_idxs)
    assert idx.shape == (B, S, W)
    rows_per_core = B // N_CORES
    in_maps = []
    for c in range(N_CORES):
        m = dict(wts)
        m["idx"] = np.ascontiguousarray(
            idx[c * rows_per_core:(c + 1) * rows_per_core]
            .reshape(TOK_PER_CORE, 16).T.astype(np.int16)
        )
        in_maps.append(m)
    return in_maps


def kernel(**inputs) -> np.ndarray:
    in_maps = make_in_maps(**inputs)
    sharded, meta = _get_runner()
    concat_in, concat_zeros = _concat_inputs(in_maps, meta)
    out_arrs = sharded(*concat_in, *concat_zeros)
    out = np.asarray(out_arrs[0])  # [8*128, 3200]
    rows_per_core = B // N_CORES
    parts = []
    for c in range(N_CORES):
        oc = out[c * 128:(c + 1) * 128]  # [128, 3200]
        parts.append(oc.T.reshape(rows_per_core, S, HID))
    return np.ascontiguousarray(np.concatenate(parts, axis=0))


def time_kernel(inputs, repeat=(8, 25), reps=20):
    """Per-pass exec time from the slope between two repeat factors.

    Wall(R) = dispatch + hidden-overlap + R * exec, so
    exec = (wall(R2) - wall(R1)) / (R2 - R1) with interleaved sampling.
    """
    import time
    import jax
    from jax.sharding import Mesh, PartitionSpec, NamedSharding

    in_maps = make_in_maps(**inputs)
    sharded, meta = _get_runner()
    concat_in, concat_zeros = _concat_inputs(in_maps, meta)
    mesh = Mesh(np.asarray(jax.devices()[:N_CORES]), ("core",))
    shd = NamedSharding(mesh, PartitionSpec("core"))
    d_in = [jax.device_put(a, shd) for a in concat_in]
    d_zero = [jax.device_put(a, shd) for a in concat_zeros]

    r1, r2 = repeat
    fns = []
    for r in (r1, r2):
        key = ("rep", r)
        if key not in _cache:
            nc_r = build_program(repeat=r)
            _cache[key] = _make_sharded(nc_r)
        fns.append(_cache[key][0])
    fn_1, fn_2 = fns

    def timed(fn, args):
        t0 = time.perf_counter()
        out = fn(*args)
        jax.block_until_ready(out)
        return time.perf_counter() - t0

    timed(fn_1, (*d_in, *d_zero))
    timed(fn_2, (*d_in, *d_zero))
    diffs, t1s = [], []
    for _ in range(reps):
        a = timed(fn_1, (*d_in, *d_zero))
        b = timed(fn_2, (*d_in, *d_zero))
        t1s.append(a)
        diffs.append(b - a)
    diffs.sort()
    t1s.sort()
    med = diffs[len(diffs) // 2]
    per_pass = med / (r2 - r1)
    return per_pass * 1e9, t1s[len(t1s) // 2] * 1e9, med * 1e9
